# revision 17
# baseline (speedup 1.0000x reference)
"""YOLOv5-style ComputeLoss on 8 Trainium2 NeuronCores.

v3 — single-activation-table exp-route device kernel.

Host (numpy): builds every index array, gathers the <=5 matched rows per
target itself (one fancy-index over ~15k entries), packs only the ACTIVE
slot entries densely (~1.8k entries/core -> T=15 columns of 128), and
uploads one bf16 blob [negated box logits | objectness plane | class
logits] + a small f32 target-geometry tensor per core (~0.6MB total).

Device per core (SPMD):
  * one manual ACT-table load (natural_log_exp_and_others serves both
    Exp and Ln; the auto-inserter would greedily flip-flop tables)
  * exp over the whole blob (the box slice is host-negated so a single
    scale=+1 pass yields exp(-box) there)
  * ln(1+e) over the obj/cls slices -> softplus; DVE reduces: per-level
    objectness sums, per-entry class-BCE sums
  * box sigmoid = 1/(1+exp(-x)) via DVE add+reciprocal, then the full
    GIoU chain on [128, 2T]
  * inputs DMA'd via three parallel triggers (sync/gpsimd/tensor),
    outputs in two overlapping DMAs
Host finalize: exact scatter-max dedup for objectness targets, masked
scalar reductions, final loss weighting (float64).
"""
import contextlib

import ml_dtypes
import numpy as np

import concourse.bacc as bacc
import concourse.mybir as mybir
import concourse.tile as tile
from concourse import bass_utils
from concourse.hw_specs import get_activation_tables

NCLS = 80
ANCHOR_T = 4.0
BALANCE = (4.0, 1.0, 0.4)
HYP_BOX, HYP_CLS, HYP_OBJ = 0.05, 0.5, 1.0
_ANCHORS_PX = np.array([[10, 13, 16, 30, 33, 23],
                        [30, 61, 62, 45, 59, 119],
                        [116, 90, 156, 198, 373, 326]],
                       np.float32).reshape(3, 3, 2)
_STRIDES = np.array([8., 16., 32.], np.float32)
ANCHORS = _ANCHORS_PX / _STRIDES[:, None, None]     # [3,3,2] feature scale
LEVEL_HW = [(80, 80), (40, 40), (20, 20)]
N_IMG = 32
N_CORES = 8
IMG_PER_CORE = N_IMG // N_CORES
A = 3
EPS = 1e-7
OBJ_COLS = [600, 150, 38]     # ceil(4*3*H*W/128) per level (level2 padded)
OBJ_W = sum(OBJ_COLS)         # 788
OBJ_PAD_VAL = -100.0          # exp(-100) == 0 in bf16 -> softplus contrib 0
F32 = mybir.dt.float32
BF16 = mybir.dt.bfloat16
BF16_NP = ml_dtypes.bfloat16

# slot order: C, L, T, R, B -> (dy, dx)
SLOT_D = np.array([[0, 0], [0, -1], [-1, 0], [0, 1], [1, 0]], np.int64)


# --------------------------------------------------------------------------
# host preprocessing
# --------------------------------------------------------------------------

def _build_level(targets, lvl):
    H, W = LEVEL_HW[lvl]
    M = targets.shape[0]
    gain = np.array([1, 1, W, H, W, H], np.float32)
    t = (targets * gain).astype(np.float32)
    anc = ANCHORS[lvl]
    with np.errstate(divide='ignore', invalid='ignore'):
        r = anc[:, None, :] / t[None, :, 4:6]
        bmask = np.max(np.maximum(r, 1.0 / r), axis=2) < ANCHOR_T   # [3, M]
    bmask = bmask & np.isfinite(t[:, 4:6]).all(1)[None, :]

    img = np.clip(targets[:, 0].astype(np.int32), 0, N_IMG - 1)
    cls_id = np.clip(targets[:, 1].astype(np.int32), 0, NCLS - 1)
    cx, cy = t[:, 2], t[:, 3]
    remx, remy = cx % 1.0, cy % 1.0
    gx0 = np.floor(cx).astype(np.int64)
    gy0 = np.floor(cy).astype(np.int64)

    sl_ok = np.stack([
        np.ones(M, bool),
        (remx < 0.5) & (cx > 1.0),
        (remy < 0.5) & (cy > 1.0),
        (remx > 0.5) & (cx < W - 1.0),
        (remy > 0.5) & (cy < H - 1.0),
    ])
    cellx = np.clip(gx0[None, :] + SLOT_D[:, 1][:, None], 0, W - 1)
    celly = np.clip(gy0[None, :] + SLOT_D[:, 0][:, None], 0, H - 1)
    offs = np.array([[0., 0.], [0.5, 0.], [0., 0.5], [-0.5, 0.], [0., -0.5]],
                    np.float32)
    offx = cx[None, :] - np.floor(cx[None, :] - offs[:, 0][:, None])
    offy = cy[None, :] - np.floor(cy[None, :] - offs[:, 1][:, None])
    return dict(H=H, W=W, bmask=bmask, img=img, cls_id=cls_id,
                tw=t[:, 4], th=t[:, 5], sl_ok=sl_ok, cellx=cellx,
                celly=celly, offx=offx, offy=offy, anc=anc)


class _Prep:
    """Builds the dense per-core device inputs + finalize metadata."""

    def __init__(self, targets, p_list):
        targets = np.asarray(targets, np.float32)
        cols = {k: [] for k in ('lvl', 'img', 'a', 'cy', 'cx', 'ox', 'oy',
                                'tw', 'th', 'cls')}
        rows_parts = []
        self.lv_sizes = []
        for lvl in range(3):
            L = _build_level(targets, lvl)
            aa, mm = np.nonzero(L['bmask'])
            n_lvl = 0
            e_img, e_a, e_cy, e_cx = [], [], [], []
            for s in range(5):
                sel = L['sl_ok'][s, mm]
                asel, msel = aa[sel], mm[sel]
                n = len(asel)
                n_lvl += n
                e_img.append(L['img'][msel])
                e_a.append(asel)
                e_cy.append(L['celly'][s, msel])
                e_cx.append(L['cellx'][s, msel])
                cols['ox'].append(L['offx'][s, msel])
                cols['oy'].append(L['offy'][s, msel])
                cols['tw'].append(L['tw'][msel])
                cols['th'].append(L['th'][msel])
                cols['cls'].append(L['cls_id'][msel])
                cols['lvl'].append(np.full(n, lvl, np.int64))
            e_img = np.concatenate(e_img)
            e_a = np.concatenate(e_a)
            e_cy = np.concatenate(e_cy)
            e_cx = np.concatenate(e_cx)
            cols['img'].append(e_img)
            cols['a'].append(e_a)
            cols['cy'].append(e_cy)
            cols['cx'].append(e_cx)
            self.lv_sizes.append(n_lvl)
            H, W = LEVEL_HW[lvl]
            pr = p_list[lvl].reshape(N_IMG, A, 5 + NCLS, H, W)
            rows_parts.append(pr[e_img, e_a, :, e_cy, e_cx])   # [n_lvl, 85]

        self.e = {k: np.concatenate(v) for k, v in cols.items()}
        rows = np.concatenate(rows_parts, axis=0)              # [ntot, 85]
        self.ntot = rows.shape[0]
        self.T = max(1, -(-self.ntot // (N_CORES * 128)))
        self.E = self.T * 128
        T = self.T

        e = self.e
        self.x_obj = rows[:, 4].astype(np.float64)
        self.x_tgt = rows[np.arange(self.ntot), 5 + e['cls']].astype(np.float64)
        anc2 = 2.0 * ANCHORS[e['lvl'], e['a']]                 # [ntot, 2]
        # +0.5 shift: device uses pxy = 2*sigma (not 2*sigma - 0.5); GIoU is
        # translation-invariant so the target corners absorb the shift.
        tc1 = np.stack([e['ox'] - e['tw'] * 0.5 + 0.5,
                        e['oy'] - e['th'] * 0.5 + 0.5], axis=1)
        tc2 = np.stack([e['ox'] + e['tw'] * 0.5 + 0.5,
                        e['oy'] + e['th'] * 0.5 + 0.5], axis=1)
        tarea = (e['tw'] * e['th'] + EPS)[:, None]

        self.negbox4 = self._pack(-rows[:, 0:4], 0.0).astype(BF16_NP)
        self.cls80 = self._pack(rows[:, 5:85], 0.0).astype(BF16_NP)
        rdp = [self._pack(tc1, 0.0), self._pack(tc2, 1.0),
               self._pack(anc2.astype(np.float32), 1.0),
               self._pack(tarea, 1.0)]
        self.rdp = np.concatenate(rdp, axis=2)                 # [8,128,7T]
        # out layout: [giou (T) | obj sums (3) | cls sums (T)]
        self.OUTW = 2 * T + 3

    def _pack(self, arr, pad_val):
        """[ntot, w] -> [8, 128, T*w]; entry j of core c at p=j%128,t=j//128."""
        w = arr.shape[1]
        full = np.full((N_CORES * self.E, w), pad_val, np.float32)
        full[:self.ntot] = arr
        return np.ascontiguousarray(
            full.reshape(N_CORES, self.T, 128, w).transpose(0, 2, 1, 3)
            .reshape(N_CORES, 128, self.T * w))

    def _unpack(self, dev_cols):
        """[8, 128, T] device outputs -> [ntot] in global entry order."""
        return (dev_cols.transpose(0, 2, 1).reshape(N_CORES * self.E)
                [:self.ntot].astype(np.float64))

    def build_blob(self, p_list, c):
        """[negbox4 | obj plane | cls] bf16 [128, 4T + OBJ_W + 80T]."""
        objs = [self.negbox4[c]]
        for lvl in range(3):
            H, W = LEVEL_HW[lvl]
            p = p_list[lvl][c * IMG_PER_CORE:(c + 1) * IMG_PER_CORE]
            ob = np.ascontiguousarray(
                p.reshape(IMG_PER_CORE, A, 5 + NCLS, H, W)[:, :, 4]).reshape(-1)
            need = 128 * OBJ_COLS[lvl]
            if len(ob) < need:
                ob = np.concatenate(
                    [ob, np.full(need - len(ob), OBJ_PAD_VAL, np.float32)])
            objs.append(ob.reshape(128, OBJ_COLS[lvl]).astype(BF16_NP))
        objs.append(self.cls80[c])
        return np.concatenate(objs, axis=1)

    def finalize(self, outs):
        T = self.T
        out3 = np.stack(outs)                                  # [8,128,2T+3]
        gp = self._unpack(out3[:, :, 0:T])                     # iou + un/ca
        cls_sum = self._unpack(out3[:, :, T + 3:2 * T + 3])
        e = self.e
        total = 0.0
        off = 0
        for lvl in range(3):
            n = self.lv_sizes[lvl]
            sl = slice(off, off + n)
            off += n
            H, W = LEVEL_HW[lvl]
            cnt = max(float(n), 1.0)
            lbox = np.sum(2.0 - gp[sl]) / cnt
            lcls = (np.sum(cls_sum[sl]) - np.sum(self.x_tgt[sl])) / (cnt * NCLS)
            s_obj = float(out3[:, :, T + lvl].sum(dtype=np.float64))
            # scatter-max dedup of clamped giou into objectness targets
            corr = 0.0
            if n:
                G = gp[sl] - 1.0
                fk = (((e['img'][sl] * A + e['a'][sl]) * H + e['cy'][sl]) * W
                      + e['cx'][sl])
                order = np.argsort(fk, kind='stable')
                fk_s = fk[order]
                vv = np.clip(G, 0.0, None)[order]
                xx = self.x_obj[sl][order]
                _, start = np.unique(fk_s, return_index=True)
                ymax = np.maximum.reduceat(vv, start)
                corr = np.sum(ymax * xx[start])
            count = N_IMG * A * H * W
            lobj = (s_obj - corr) / count
            total += (HYP_BOX * lbox + HYP_CLS * lcls
                      + HYP_OBJ * BALANCE[lvl] * lobj)
        return np.float32(total * N_IMG)


# --------------------------------------------------------------------------
# device kernel
# --------------------------------------------------------------------------

def _exp_ln_table_id(nc):
    tabs = get_activation_tables(nc.m.arch)
    act = mybir.ActivationFunctionType
    for i, funcs in enumerate(tabs.values()):
        if act.Exp in funcs and act.Ln in funcs:
            return i
    return None


def _build_bass(T):
    nc = bacc.Bacc('TRN2', debug=False, num_devices=N_CORES)
    BW = 4 * T + OBJ_W + 80 * T          # blob cols: negbox | obj | cls
    ob0 = 4 * T                          # obj slice start
    cb0 = ob0 + OBJ_W                    # cls slice start
    blob_d = nc.dram_tensor('blob', [128, BW], BF16, kind='ExternalInput')
    rdp_d = nc.dram_tensor('rdp', [128, 7 * T], F32, kind='ExternalInput')
    out_d = nc.dram_tensor('out', [128, 2 * T + 3], F32, kind='ExternalOutput')

    with tile.TileContext(nc) as tc:
        with contextlib.ExitStack() as ctx:
            pool = ctx.enter_context(tc.tile_pool(name='sbuf', bufs=1))
            tt = mybir.AluOpType
            act = mybir.ActivationFunctionType

            blob_t = pool.tile([128, BW], BF16)
            # transfers serialize on the DMA engine in trigger order, so
            # issue box+obj first (unblocks ACT earliest), then cls
            nc.sync.dma_start(blob_t[:, 0:cb0], blob_d.ap()[:, 0:cb0])
            nc.sync.dma_start(blob_t[:, cb0:BW], blob_d.ap()[:, cb0:BW])
            rdp_t = pool.tile([128, 7 * T], F32)
            nc.gpsimd.dma_start(rdp_t[:], rdp_d.ap())
            out_t = pool.tile([128, 2 * T + 3], F32)

            tc1 = rdp_t[:, 0:2 * T]
            tc2 = rdp_t[:, 2 * T:4 * T]
            awh2 = rdp_t[:, 4 * T:6 * T]
            tarea = rdp_t[:, 6 * T:7 * T]

            # ---- scalar engine: preload the exp+ln table once, then
            # exp over the blob and ln(1+e) over the obj/cls slices.
            tab = _exp_ln_table_id(nc)
            if tab is not None:
                nc.scalar.add_instruction(mybir.InstLoadActFuncSet(
                    act_func_set_id=tab, name=nc.get_next_instruction_name(),
                    engine=mybir.EngineType.Activation, ins=[], outs=[]))
            pe = pool.tile([128, BW], BF16)
            nc.scalar.activation(pe[:, 0:ob0], blob_t[:, 0:ob0], act.Exp)
            nc.scalar.activation(pe[:, ob0:cb0], blob_t[:, ob0:cb0], act.Exp)
            nc.scalar.activation(pe[:, cb0:BW], blob_t[:, cb0:BW], act.Exp)
            lno = pool.tile([128, OBJ_W], BF16)
            nc.scalar.activation(lno[:], pe[:, ob0:cb0], act.Ln, bias=1.0)
            CH1 = 8 * 80                   # cls ln/reduce chunk split
            lnc = pool.tile([128, 80 * T], BF16)
            nc.scalar.activation(lnc[:, 0:CH1], pe[:, cb0:cb0 + CH1],
                                 act.Ln, bias=1.0)
            nc.scalar.activation(lnc[:, CH1:80 * T], pe[:, cb0 + CH1:BW],
                                 act.Ln, bias=1.0)

            # ---- vector engine: box sigmoid, GIoU chain, reductions
            def f32t(w, tag):
                return pool.tile([128, w], F32, name=tag, tag=tag)

            def xy(ap2):
                v = ap2.rearrange('p (c e) -> p c e', e=2)
                return v[:, :, 0], v[:, :, 1]

            sd = f32t(4 * T, 'sd')     # 1 + exp(-x)
            nc.vector.tensor_scalar_add(sd[:], pe[:, 0:ob0], 1.0)
            sig = f32t(4 * T, 'sig')
            nc.vector.reciprocal(sig[:], sd[:])
            sig4 = sig[:].rearrange('p (c e) -> p c e', e=4)
            pxy = f32t(2 * T, 'pxy')   # 2*sigma (host shifted tc by +0.5)
            nc.vector.tensor_scalar_mul(pxy[:].rearrange('p (c e) -> p c e',
                                                         e=2),
                                        sig4[:, :, 0:2], 2.0)
            sq = f32t(2 * T, 'sq')
            nc.vector.tensor_tensor(out=sq[:].rearrange('p (c e) -> p c e',
                                                        e=2),
                                    in0=sig4[:, :, 2:4], in1=sig4[:, :, 2:4],
                                    op=tt.mult)
            hwh = f32t(2 * T, 'hwh')   # pwh/2 = 2*anc*sig^2
            nc.vector.tensor_tensor(out=hwh[:], in0=sq[:], in1=awh2,
                                    op=tt.mult)
            b1 = f32t(2 * T, 'b1')
            nc.vector.scalar_tensor_tensor(out=b1[:], in0=hwh[:], scalar=-1.0,
                                           in1=pxy[:], op0=tt.mult, op1=tt.add)
            b2 = f32t(2 * T, 'b2')
            nc.vector.tensor_tensor(out=b2[:], in0=hwh[:], in1=pxy[:],
                                    op=tt.add)
            i1 = f32t(2 * T, 'i1')
            nc.vector.tensor_tensor(out=i1[:], in0=b1[:], in1=tc1, op=tt.max)
            i2 = f32t(2 * T, 'i2')
            nc.vector.tensor_tensor(out=i2[:], in0=b2[:], in1=tc2, op=tt.min)
            c1 = f32t(2 * T, 'c1')
            nc.vector.tensor_tensor(out=c1[:], in0=b1[:], in1=tc1, op=tt.min)
            c2 = f32t(2 * T, 'c2')
            nc.vector.tensor_tensor(out=c2[:], in0=b2[:], in1=tc2, op=tt.max)
            iw = f32t(2 * T, 'iw')
            nc.vector.tensor_tensor(out=iw[:], in0=i2[:], in1=i1[:],
                                    op=tt.subtract)
            iwc = f32t(2 * T, 'iwc')
            nc.vector.tensor_scalar_max(iwc[:], iw[:], 0.0)
            iwx, iwy = xy(iwc[:])
            inter = f32t(T, 'inter')
            nc.vector.tensor_tensor(out=inter[:], in0=iwx, in1=iwy, op=tt.mult)
            hx, hy = xy(hwh[:])
            hp = f32t(T, 'hp')
            nc.vector.tensor_tensor(out=hp[:], in0=hx, in1=hy, op=tt.mult)
            u1 = f32t(T, 'u1')        # parea + tarea = 4*hp + tarea
            nc.vector.scalar_tensor_tensor(out=u1[:], in0=hp[:], scalar=4.0,
                                           in1=tarea, op0=tt.mult, op1=tt.add)
            un = f32t(T, 'un')
            nc.vector.tensor_tensor(out=un[:], in0=u1[:], in1=inter[:],
                                    op=tt.subtract)
            ru = f32t(T, 'ru')
            nc.vector.reciprocal(ru[:], un[:])
            iou = f32t(T, 'iou')
            nc.vector.tensor_tensor(out=iou[:], in0=inter[:], in1=ru[:],
                                    op=tt.mult)
            cwh = f32t(2 * T, 'cwh')
            nc.vector.tensor_tensor(out=cwh[:], in0=c2[:], in1=c1[:],
                                    op=tt.subtract)
            cwx, cwy = xy(cwh[:])
            ca = f32t(T, 'ca')        # cw*ch (>0 strictly; eps dropped)
            nc.vector.tensor_tensor(out=ca[:], in0=cwx, in1=cwy, op=tt.mult)
            rc = f32t(T, 'rc')
            nc.vector.reciprocal(rc[:], ca[:])
            q = f32t(T, 'q')
            nc.vector.tensor_tensor(out=q[:], in0=un[:], in1=rc[:], op=tt.mult)
            # giou = iou - (ca-un)/ca = (iou + un/ca) - 1 ; host subtracts 1
            nc.vector.tensor_tensor(out=out_t[:, 0:T], in0=iou[:], in1=q[:],
                                    op=tt.add)

            # ---- pairwise folds on the idle gpsimd engine cut the DVE
            # reduce work 2x (obj) / 4x (cls): fold[i] = x[2i] + x[2i+1]
            def fold(dst, src_ap, w2):
                v = src_ap.rearrange('p (b e) -> p b e', e=2)
                nc.gpsimd.tensor_tensor(out=dst[:, 0:w2], in0=v[:, :, 0],
                                        in1=v[:, :, 1], op=tt.add)

            lnof = pool.tile([128, OBJ_W // 2], BF16)
            fold(lnof, lno[:], OBJ_W // 2)
            # per-level objectness softplus sums -> out[:, T:T+3]
            o = 0
            for lvl in range(3):
                w = OBJ_COLS[lvl] // 2
                nc.vector.reduce_sum(out_t[:, T + lvl:T + lvl + 1],
                                     lnof[:, o:o + w],
                                     axis=mybir.AxisListType.X)
                o += w
            # first output: giou + obj sums, overlaps the cls tail
            nc.sync.dma_start(out_d.ap()[:, 0:T + 3], out_t[:, 0:T + 3])

            # per-entry cls softplus sums -> out[:, T+3:2T+3]
            CH2 = 80 * T - CH1
            lf1 = pool.tile([128, 80 * T // 2], BF16)
            lf2 = pool.tile([128, 80 * T // 4], BF16)
            fold(lf1, lnc[:, 0:CH1], CH1 // 2)
            fold(lf2, lf1[:, 0:CH1 // 2], CH1 // 4)
            fold(lf1[:, CH1 // 2:], lnc[:, CH1:80 * T], CH2 // 2)
            fold(lf2[:, CH1 // 4:], lf1[:, CH1 // 2:], CH2 // 4)
            nc.vector.reduce_sum(
                out_t[:, T + 3:T + 3 + CH1 // 80],
                lf2[:, 0:CH1 // 4].rearrange('p (b e) -> p b e', e=20),
                axis=mybir.AxisListType.X)
            nc.vector.reduce_sum(
                out_t[:, T + 3 + CH1 // 80:2 * T + 3],
                lf2[:, CH1 // 4:].rearrange('p (b e) -> p b e', e=20),
                axis=mybir.AxisListType.X)
            nc.sync.dma_start(out_d.ap()[:, T + 3:2 * T + 3],
                              out_t[:, T + 3:2 * T + 3])
    nc.compile()
    return nc


# --------------------------------------------------------------------------
# entry point
# --------------------------------------------------------------------------

def kernel(p0, p1, p2, targets):
    p0 = np.asarray(p0, np.float32)
    p1 = np.asarray(p1, np.float32)
    p2 = np.asarray(p2, np.float32)
    targets = np.asarray(targets, np.float32)
    p_list = [p0, p1, p2]
    prep = _Prep(targets, p_list)
    nc = _build_bass(prep.T)

    in_maps = []
    for c in range(N_CORES):
        in_maps.append({
            'blob': prep.build_blob(p_list, c),
            'rdp': prep.rdp[c],
        })
    res = bass_utils.run_bass_kernel_spmd(nc, in_maps,
                                          core_ids=list(range(N_CORES)))
    global LAST_EXEC_NS, LAST_RESULT
    LAST_EXEC_NS = res.exec_time_ns
    LAST_RESULT = res
    outs = [res.results[c]['out'] for c in range(N_CORES)]
    return np.asarray(prep.finalize(outs), np.float32)


LAST_EXEC_NS = None
LAST_RESULT = None


# revision 26
# speedup vs baseline: 1.1182x; 1.1182x over previous
"""YOLOv5-style ComputeLoss on 8 Trainium2 NeuronCores.

v3 — single-activation-table exp-route device kernel.

Host (numpy): builds every index array, gathers the <=5 matched rows per
target itself (one fancy-index over ~15k entries), packs only the ACTIVE
slot entries densely (~1.8k entries/core -> T=15 columns of 128), and
uploads one bf16 blob [negated box logits | objectness plane | class
logits] + a small f32 target-geometry tensor per core (~0.6MB total).

Device per core (SPMD):
  * one manual ACT-table load (natural_log_exp_and_others serves both
    Exp and Ln; the auto-inserter would greedily flip-flop tables)
  * exp over the whole blob (the box slice is host-negated so a single
    scale=+1 pass yields exp(-box) there)
  * ln(1+e) over the obj/cls slices -> softplus; DVE reduces: per-level
    objectness sums, per-entry class-BCE sums
  * box sigmoid = 1/(1+exp(-x)) via DVE add+reciprocal, then the full
    GIoU chain on [128, 2T]
  * inputs DMA'd via three parallel triggers (sync/gpsimd/tensor),
    outputs in two overlapping DMAs
Host finalize: exact scatter-max dedup for objectness targets, masked
scalar reductions, final loss weighting (float64).
"""
import contextlib

import ml_dtypes
import numpy as np

import concourse.bacc as bacc
import concourse.mybir as mybir
import concourse.tile as tile
from concourse import bass_utils
from concourse.hw_specs import get_activation_tables

NCLS = 80
ANCHOR_T = 4.0
BALANCE = (4.0, 1.0, 0.4)
HYP_BOX, HYP_CLS, HYP_OBJ = 0.05, 0.5, 1.0
_ANCHORS_PX = np.array([[10, 13, 16, 30, 33, 23],
                        [30, 61, 62, 45, 59, 119],
                        [116, 90, 156, 198, 373, 326]],
                       np.float32).reshape(3, 3, 2)
_STRIDES = np.array([8., 16., 32.], np.float32)
ANCHORS = _ANCHORS_PX / _STRIDES[:, None, None]     # [3,3,2] feature scale
LEVEL_HW = [(80, 80), (40, 40), (20, 20)]
N_IMG = 32
N_CORES = 8
IMG_PER_CORE = N_IMG // N_CORES
A = 3
EPS = 1e-7
OBJ_COLS = [600, 150, 38]     # ceil(4*3*H*W/128) per level (level2 padded)
OBJ_W = sum(OBJ_COLS)         # 788
OBJ_PAD_VAL = -100.0          # exp(-100) == 0 in bf16 -> softplus contrib 0
F32 = mybir.dt.float32
BF16 = mybir.dt.bfloat16
BF16_NP = ml_dtypes.bfloat16
FP8 = mybir.dt.float8e4
FP8_NP = mybir.dt.np(mybir.dt.float8e4)

# slot order: C, L, T, R, B -> (dy, dx)
SLOT_D = np.array([[0, 0], [0, -1], [-1, 0], [0, 1], [1, 0]], np.int64)


# --------------------------------------------------------------------------
# host preprocessing
# --------------------------------------------------------------------------

def _build_level(targets, lvl):
    H, W = LEVEL_HW[lvl]
    M = targets.shape[0]
    gain = np.array([1, 1, W, H, W, H], np.float32)
    t = (targets * gain).astype(np.float32)
    anc = ANCHORS[lvl]
    with np.errstate(divide='ignore', invalid='ignore'):
        r = anc[:, None, :] / t[None, :, 4:6]
        bmask = np.max(np.maximum(r, 1.0 / r), axis=2) < ANCHOR_T   # [3, M]
    bmask = bmask & np.isfinite(t[:, 4:6]).all(1)[None, :]

    img = np.clip(targets[:, 0].astype(np.int32), 0, N_IMG - 1)
    cls_id = np.clip(targets[:, 1].astype(np.int32), 0, NCLS - 1)
    cx, cy = t[:, 2], t[:, 3]
    remx, remy = cx % 1.0, cy % 1.0
    gx0 = np.floor(cx).astype(np.int64)
    gy0 = np.floor(cy).astype(np.int64)

    sl_ok = np.stack([
        np.ones(M, bool),
        (remx < 0.5) & (cx > 1.0),
        (remy < 0.5) & (cy > 1.0),
        (remx > 0.5) & (cx < W - 1.0),
        (remy > 0.5) & (cy < H - 1.0),
    ])
    cellx = np.clip(gx0[None, :] + SLOT_D[:, 1][:, None], 0, W - 1)
    celly = np.clip(gy0[None, :] + SLOT_D[:, 0][:, None], 0, H - 1)
    offs = np.array([[0., 0.], [0.5, 0.], [0., 0.5], [-0.5, 0.], [0., -0.5]],
                    np.float32)
    offx = cx[None, :] - np.floor(cx[None, :] - offs[:, 0][:, None])
    offy = cy[None, :] - np.floor(cy[None, :] - offs[:, 1][:, None])
    return dict(H=H, W=W, bmask=bmask, img=img, cls_id=cls_id,
                tw=t[:, 4], th=t[:, 5], sl_ok=sl_ok, cellx=cellx,
                celly=celly, offx=offx, offy=offy, anc=anc)


class _Prep:
    """Builds the dense per-core device inputs + finalize metadata."""

    def __init__(self, targets, p_list):
        targets = np.asarray(targets, np.float32)
        cols = {k: [] for k in ('lvl', 'img', 'a', 'cy', 'cx', 'ox', 'oy',
                                'tw', 'th', 'cls')}
        rows_parts = []
        self.lv_sizes = []
        for lvl in range(3):
            L = _build_level(targets, lvl)
            aa, mm = np.nonzero(L['bmask'])
            n_lvl = 0
            e_img, e_a, e_cy, e_cx = [], [], [], []
            for s in range(5):
                sel = L['sl_ok'][s, mm]
                asel, msel = aa[sel], mm[sel]
                n = len(asel)
                n_lvl += n
                e_img.append(L['img'][msel])
                e_a.append(asel)
                e_cy.append(L['celly'][s, msel])
                e_cx.append(L['cellx'][s, msel])
                cols['ox'].append(L['offx'][s, msel])
                cols['oy'].append(L['offy'][s, msel])
                cols['tw'].append(L['tw'][msel])
                cols['th'].append(L['th'][msel])
                cols['cls'].append(L['cls_id'][msel])
                cols['lvl'].append(np.full(n, lvl, np.int64))
            e_img = np.concatenate(e_img)
            e_a = np.concatenate(e_a)
            e_cy = np.concatenate(e_cy)
            e_cx = np.concatenate(e_cx)
            cols['img'].append(e_img)
            cols['a'].append(e_a)
            cols['cy'].append(e_cy)
            cols['cx'].append(e_cx)
            self.lv_sizes.append(n_lvl)
            H, W = LEVEL_HW[lvl]
            pr = p_list[lvl].reshape(N_IMG, A, 5 + NCLS, H, W)
            rows_parts.append(pr[e_img, e_a, :, e_cy, e_cx])   # [n_lvl, 85]

        self.e = {k: np.concatenate(v) for k, v in cols.items()}
        rows = np.concatenate(rows_parts, axis=0)              # [ntot, 85]
        self.ntot = rows.shape[0]
        self.T = max(1, -(-self.ntot // (N_CORES * 128)))
        self.E = self.T * 128
        T = self.T

        e = self.e
        self.x_obj = rows[:, 4].astype(np.float64)
        self.x_tgt = rows[np.arange(self.ntot), 5 + e['cls']].astype(np.float64)
        anc2 = 2.0 * ANCHORS[e['lvl'], e['a']]                 # [ntot, 2]
        # +0.5 shift: device uses pxy = 2*sigma (not 2*sigma - 0.5); GIoU is
        # translation-invariant so the target corners absorb the shift.
        tc1 = np.stack([e['ox'] - e['tw'] * 0.5 + 0.5,
                        e['oy'] - e['th'] * 0.5 + 0.5], axis=1)
        tc2 = np.stack([e['ox'] + e['tw'] * 0.5 + 0.5,
                        e['oy'] + e['th'] * 0.5 + 0.5], axis=1)
        tarea = (e['tw'] * e['th'] + EPS)[:, None]

        self.negbox4 = self._pack(-rows[:, 0:4], 0.0).astype(BF16_NP)
        self.cls80 = self._pack(rows[:, 5:85], 0.0).astype(FP8_NP)
        rdp = [self._pack(tc1, 0.0), self._pack(tc2, 1.0),
               self._pack(anc2.astype(np.float32), 1.0),
               self._pack(tarea, 1.0)]
        self.rdp = np.concatenate(rdp, axis=2)                 # [8,128,7T]
        # out layout: [giou (T) | obj sums (3) | cls sums (T)]
        self.OUTW = 2 * T + 3

    def _pack(self, arr, pad_val):
        """[ntot, w] -> [8, 128, T*w]; entry j of core c at p=j%128,t=j//128."""
        w = arr.shape[1]
        full = np.full((N_CORES * self.E, w), pad_val, np.float32)
        full[:self.ntot] = arr
        return np.ascontiguousarray(
            full.reshape(N_CORES, self.T, 128, w).transpose(0, 2, 1, 3)
            .reshape(N_CORES, 128, self.T * w))

    def _unpack(self, dev_cols):
        """[8, 128, T] device outputs -> [ntot] in global entry order."""
        return (dev_cols.transpose(0, 2, 1).reshape(N_CORES * self.E)
                [:self.ntot].astype(np.float64))

    def build_objcls(self, p_list, c):
        """[obj plane | cls] fp8 [128, OBJ_W + 80T]."""
        objs = []
        for lvl in range(3):
            H, W = LEVEL_HW[lvl]
            p = p_list[lvl][c * IMG_PER_CORE:(c + 1) * IMG_PER_CORE]
            ob = np.ascontiguousarray(
                p.reshape(IMG_PER_CORE, A, 5 + NCLS, H, W)[:, :, 4]).reshape(-1)
            need = 128 * OBJ_COLS[lvl]
            if len(ob) < need:
                ob = np.concatenate(
                    [ob, np.full(need - len(ob), OBJ_PAD_VAL, np.float32)])
            objs.append(ob.reshape(128, OBJ_COLS[lvl]).astype(FP8_NP))
        objs.append(self.cls80[c])
        return np.concatenate(objs, axis=1)

    def finalize(self, outs):
        T = self.T
        out3 = np.stack(outs)                                  # [8,128,2T+3]
        gp = self._unpack(out3[:, :, 0:T])                     # iou + un/ca
        cls_sum = self._unpack(out3[:, :, T + 3:2 * T + 3])
        e = self.e
        total = 0.0
        off = 0
        for lvl in range(3):
            n = self.lv_sizes[lvl]
            sl = slice(off, off + n)
            off += n
            H, W = LEVEL_HW[lvl]
            cnt = max(float(n), 1.0)
            lbox = np.sum(2.0 - gp[sl]) / cnt
            lcls = (np.sum(cls_sum[sl]) - np.sum(self.x_tgt[sl])) / (cnt * NCLS)
            s_obj = float(out3[:, :, T + lvl].sum(dtype=np.float64))
            # scatter-max dedup of clamped giou into objectness targets
            corr = 0.0
            if n:
                G = gp[sl] - 1.0
                fk = (((e['img'][sl] * A + e['a'][sl]) * H + e['cy'][sl]) * W
                      + e['cx'][sl])
                order = np.argsort(fk, kind='stable')
                fk_s = fk[order]
                vv = np.clip(G, 0.0, None)[order]
                xx = self.x_obj[sl][order]
                _, start = np.unique(fk_s, return_index=True)
                ymax = np.maximum.reduceat(vv, start)
                corr = np.sum(ymax * xx[start])
            count = N_IMG * A * H * W
            lobj = (s_obj - corr) / count
            total += (HYP_BOX * lbox + HYP_CLS * lcls
                      + HYP_OBJ * BALANCE[lvl] * lobj)
        return np.float32(total * N_IMG)


# --------------------------------------------------------------------------
# device kernel
# --------------------------------------------------------------------------

def _exp_ln_table_id(nc):
    tabs = get_activation_tables(nc.m.arch)
    act = mybir.ActivationFunctionType
    for i, funcs in enumerate(tabs.values()):
        if act.Exp in funcs and act.Ln in funcs:
            return i
    return None


def _build_bass(T):
    nc = bacc.Bacc('TRN2', debug=False, num_devices=N_CORES)
    BW = 4 * T + OBJ_W + 80 * T          # exp cols: negbox | obj | cls
    ob0 = 4 * T                          # obj slice start
    cb0 = ob0 + OBJ_W                    # cls slice start
    OCW = OBJ_W + 80 * T
    nb_d = nc.dram_tensor('negbox', [128, 4 * T], BF16, kind='ExternalInput')
    oc_d = nc.dram_tensor('objcls', [128, OCW], FP8, kind='ExternalInput')
    rdp_d = nc.dram_tensor('rdp', [128, 7 * T], F32, kind='ExternalInput')
    out_d = nc.dram_tensor('out', [128, 2 * T + 3], F32, kind='ExternalOutput')

    with tile.TileContext(nc) as tc:
        with contextlib.ExitStack() as ctx:
            pool = ctx.enter_context(tc.tile_pool(name='sbuf', bufs=1))
            tt = mybir.AluOpType
            act = mybir.ActivationFunctionType

            nb_t = pool.tile([128, 4 * T], BF16)
            oc_t = pool.tile([128, OCW], FP8)
            # transfers serialize on the DMA engine in trigger order, so
            # issue box, then obj, then cls (ACT consumes in that order)
            nc.sync.dma_start(nb_t[:], nb_d.ap())
            nc.sync.dma_start(oc_t[:, 0:OBJ_W], oc_d.ap()[:, 0:OBJ_W])
            nc.sync.dma_start(oc_t[:, OBJ_W:OCW], oc_d.ap()[:, OBJ_W:OCW])
            rdp_t = pool.tile([128, 7 * T], F32)
            nc.gpsimd.dma_start(rdp_t[:], rdp_d.ap())
            out_t = pool.tile([128, 2 * T + 3], F32)

            tc1 = rdp_t[:, 0:2 * T]
            tc2 = rdp_t[:, 2 * T:4 * T]
            awh2 = rdp_t[:, 4 * T:6 * T]
            tarea = rdp_t[:, 6 * T:7 * T]

            # ---- scalar engine: preload the exp+ln table once, then
            # exp over the blob and ln(1+e) over the obj/cls slices.
            tab = _exp_ln_table_id(nc)
            if tab is not None:
                nc.scalar.add_instruction(mybir.InstLoadActFuncSet(
                    act_func_set_id=tab, name=nc.get_next_instruction_name(),
                    engine=mybir.EngineType.Activation, ins=[], outs=[]))
            pe = pool.tile([128, BW], BF16)
            nc.scalar.activation(pe[:, 0:ob0], nb_t[:], act.Exp)
            nc.scalar.activation(pe[:, ob0:cb0], oc_t[:, 0:OBJ_W], act.Exp)
            nc.scalar.activation(pe[:, cb0:BW], oc_t[:, OBJ_W:OCW], act.Exp)
            lno = pool.tile([128, OBJ_W], BF16)
            nc.scalar.activation(lno[:], pe[:, ob0:cb0], act.Ln, bias=1.0)
            CH1 = 8 * 80                   # cls ln/reduce chunk split
            lnc = pool.tile([128, 80 * T], BF16)
            nc.scalar.activation(lnc[:, 0:CH1], pe[:, cb0:cb0 + CH1],
                                 act.Ln, bias=1.0)
            nc.scalar.activation(lnc[:, CH1:80 * T], pe[:, cb0 + CH1:BW],
                                 act.Ln, bias=1.0)

            # ---- vector engine: box sigmoid, GIoU chain, reductions
            def f32t(w, tag):
                return pool.tile([128, w], F32, name=tag, tag=tag)

            def xy(ap2):
                v = ap2.rearrange('p (c e) -> p c e', e=2)
                return v[:, :, 0], v[:, :, 1]

            sd = f32t(4 * T, 'sd')     # 1 + exp(-x)
            nc.vector.tensor_scalar_add(sd[:], pe[:, 0:ob0], 1.0)
            sig = f32t(4 * T, 'sig')
            nc.vector.reciprocal(sig[:], sd[:])
            sig4 = sig[:].rearrange('p (c e) -> p c e', e=4)
            sq = f32t(2 * T, 'sq')
            nc.vector.tensor_tensor(out=sq[:].rearrange('p (c e) -> p c e',
                                                        e=2),
                                    in0=sig4[:, :, 2:4], in1=sig4[:, :, 2:4],
                                    op=tt.mult)
            hwh = f32t(2 * T, 'hwh')   # pwh/2 = 2*anc*sig^2
            nc.vector.tensor_tensor(out=hwh[:], in0=sq[:], in1=awh2,
                                    op=tt.mult)
            # pxy = 2*sigma folded into both corner ops (host shifted tc +0.5)
            sxy = sig4[:, :, 0:2]
            b1 = f32t(2 * T, 'b1')
            nc.vector.scalar_tensor_tensor(out=b1[:].rearrange(
                                               'p (c e) -> p c e', e=2),
                                           in0=sxy, scalar=2.0,
                                           in1=hwh[:].rearrange(
                                               'p (c e) -> p c e', e=2),
                                           op0=tt.mult, op1=tt.subtract)
            b2 = f32t(2 * T, 'b2')
            nc.vector.scalar_tensor_tensor(out=b2[:].rearrange(
                                               'p (c e) -> p c e', e=2),
                                           in0=sxy, scalar=2.0,
                                           in1=hwh[:].rearrange(
                                               'p (c e) -> p c e', e=2),
                                           op0=tt.mult, op1=tt.add)
            i1 = f32t(2 * T, 'i1')
            nc.vector.tensor_tensor(out=i1[:], in0=b1[:], in1=tc1, op=tt.max)
            i2 = f32t(2 * T, 'i2')
            nc.vector.tensor_tensor(out=i2[:], in0=b2[:], in1=tc2, op=tt.min)
            c1 = f32t(2 * T, 'c1')
            nc.vector.tensor_tensor(out=c1[:], in0=b1[:], in1=tc1, op=tt.min)
            c2 = f32t(2 * T, 'c2')
            nc.vector.tensor_tensor(out=c2[:], in0=b2[:], in1=tc2, op=tt.max)
            iw = f32t(2 * T, 'iw')
            nc.vector.tensor_tensor(out=iw[:], in0=i2[:], in1=i1[:],
                                    op=tt.subtract)
            iwc = f32t(2 * T, 'iwc')
            nc.vector.tensor_scalar_max(iwc[:], iw[:], 0.0)
            iwx, iwy = xy(iwc[:])
            inter = f32t(T, 'inter')
            nc.vector.tensor_tensor(out=inter[:], in0=iwx, in1=iwy, op=tt.mult)
            hx, hy = xy(hwh[:])
            hp = f32t(T, 'hp')
            nc.vector.tensor_tensor(out=hp[:], in0=hx, in1=hy, op=tt.mult)
            u1 = f32t(T, 'u1')        # parea + tarea = 4*hp + tarea
            nc.vector.scalar_tensor_tensor(out=u1[:], in0=hp[:], scalar=4.0,
                                           in1=tarea, op0=tt.mult, op1=tt.add)
            un = f32t(T, 'un')
            nc.vector.tensor_tensor(out=un[:], in0=u1[:], in1=inter[:],
                                    op=tt.subtract)
            ru = f32t(T, 'ru')
            nc.vector.reciprocal(ru[:], un[:])
            iou = f32t(T, 'iou')
            nc.vector.tensor_tensor(out=iou[:], in0=inter[:], in1=ru[:],
                                    op=tt.mult)
            cwh = f32t(2 * T, 'cwh')
            nc.vector.tensor_tensor(out=cwh[:], in0=c2[:], in1=c1[:],
                                    op=tt.subtract)
            cwx, cwy = xy(cwh[:])
            ca = f32t(T, 'ca')        # cw*ch (>0 strictly; eps dropped)
            nc.vector.tensor_tensor(out=ca[:], in0=cwx, in1=cwy, op=tt.mult)
            rc = f32t(T, 'rc')
            nc.vector.reciprocal(rc[:], ca[:])
            q = f32t(T, 'q')
            nc.vector.tensor_tensor(out=q[:], in0=un[:], in1=rc[:], op=tt.mult)
            # giou = iou - (ca-un)/ca = (iou + un/ca) - 1 ; host subtracts 1
            nc.vector.tensor_tensor(out=out_t[:, 0:T], in0=iou[:], in1=q[:],
                                    op=tt.add)

            # per-level objectness softplus sums -> out[:, T:T+3]
            o = 0
            for lvl in range(3):
                w = OBJ_COLS[lvl]
                nc.vector.reduce_sum(out_t[:, T + lvl:T + lvl + 1],
                                     lno[:, o:o + w], axis=mybir.AxisListType.X)
                o += w
            # first output: giou + obj sums, overlaps the cls tail
            nc.sync.dma_start(out_d.ap()[:, 0:T + 3], out_t[:, 0:T + 3])

            # per-entry cls softplus sums -> out[:, T+3:2T+3]
            nc.vector.reduce_sum(
                out_t[:, T + 3:T + 3 + CH1 // 80],
                lnc[:, 0:CH1].rearrange('p (b e) -> p b e', e=80),
                axis=mybir.AxisListType.X)
            nc.vector.reduce_sum(
                out_t[:, T + 3 + CH1 // 80:2 * T + 3],
                lnc[:, CH1:80 * T].rearrange('p (b e) -> p b e', e=80),
                axis=mybir.AxisListType.X)
            nc.sync.dma_start(out_d.ap()[:, T + 3:2 * T + 3],
                              out_t[:, T + 3:2 * T + 3])
    nc.compile()
    return nc


# --------------------------------------------------------------------------
# entry point
# --------------------------------------------------------------------------

def kernel(p0, p1, p2, targets):
    p0 = np.asarray(p0, np.float32)
    p1 = np.asarray(p1, np.float32)
    p2 = np.asarray(p2, np.float32)
    targets = np.asarray(targets, np.float32)
    p_list = [p0, p1, p2]
    prep = _Prep(targets, p_list)
    nc = _build_bass(prep.T)

    in_maps = []
    for c in range(N_CORES):
        in_maps.append({
            'negbox': prep.negbox4[c],
            'objcls': prep.build_objcls(p_list, c),
            'rdp': prep.rdp[c],
        })
    res = bass_utils.run_bass_kernel_spmd(nc, in_maps,
                                          core_ids=list(range(N_CORES)))
    global LAST_EXEC_NS, LAST_RESULT
    LAST_EXEC_NS = res.exec_time_ns
    LAST_RESULT = res
    outs = [res.results[c]['out'] for c in range(N_CORES)]
    return np.asarray(prep.finalize(outs), np.float32)


LAST_EXEC_NS = None
LAST_RESULT = None


# revision 33
# speedup vs baseline: 1.2522x; 1.1198x over previous
"""YOLOv5-style ComputeLoss on 8 Trainium2 NeuronCores.

v3 — single-activation-table exp-route device kernel.

Host (numpy): builds every index array, gathers the <=5 matched rows per
target itself (one fancy-index over ~15k entries), packs only the ACTIVE
slot entries densely (~1.8k entries/core -> T=15 columns of 128), and
uploads one bf16 blob [negated box logits | objectness plane | class
logits] + a small f32 target-geometry tensor per core (~0.6MB total).

Device per core (SPMD):
  * one manual ACT-table load (natural_log_exp_and_others serves both
    Exp and Ln; the auto-inserter would greedily flip-flop tables)
  * exp over the whole blob (the box slice is host-negated so a single
    scale=+1 pass yields exp(-box) there)
  * ln(1+e) over the obj/cls slices -> softplus; DVE reduces: per-level
    objectness sums, per-entry class-BCE sums
  * box sigmoid = 1/(1+exp(-x)) via DVE add+reciprocal, then the full
    GIoU chain on [128, 2T]
  * inputs DMA'd via three parallel triggers (sync/gpsimd/tensor),
    outputs in two overlapping DMAs
Host finalize: exact scatter-max dedup for objectness targets, masked
scalar reductions, final loss weighting (float64).
"""
import contextlib

import ml_dtypes
import numpy as np

import concourse.bacc as bacc
import concourse.mybir as mybir
import concourse.tile as tile
from concourse import bass_utils
from concourse.hw_specs import get_activation_tables

NCLS = 80
ANCHOR_T = 4.0
BALANCE = (4.0, 1.0, 0.4)
HYP_BOX, HYP_CLS, HYP_OBJ = 0.05, 0.5, 1.0
_ANCHORS_PX = np.array([[10, 13, 16, 30, 33, 23],
                        [30, 61, 62, 45, 59, 119],
                        [116, 90, 156, 198, 373, 326]],
                       np.float32).reshape(3, 3, 2)
_STRIDES = np.array([8., 16., 32.], np.float32)
ANCHORS = _ANCHORS_PX / _STRIDES[:, None, None]     # [3,3,2] feature scale
LEVEL_HW = [(80, 80), (40, 40), (20, 20)]
N_IMG = 32
N_CORES = 8
IMG_PER_CORE = N_IMG // N_CORES
A = 3
EPS = 1e-7
OBJ_COLS = [600, 150, 38]     # ceil(4*3*H*W/128) per level (level2 padded)
OBJ_W = sum(OBJ_COLS)         # 788
OBJ_WP = 800                  # padded so the cls slice starts 32B-aligned
OBJ_PAD_VAL = -100.0          # exp(-100) == 0 in bf16 -> softplus contrib 0
F32 = mybir.dt.float32
BF16 = mybir.dt.bfloat16
BF16_NP = ml_dtypes.bfloat16
FP8 = mybir.dt.float8e4
FP8_NP = mybir.dt.np(mybir.dt.float8e4)

# slot order: C, L, T, R, B -> (dy, dx)
SLOT_D = np.array([[0, 0], [0, -1], [-1, 0], [0, 1], [1, 0]], np.int64)


# --------------------------------------------------------------------------
# host preprocessing
# --------------------------------------------------------------------------

def _build_level(targets, lvl):
    H, W = LEVEL_HW[lvl]
    M = targets.shape[0]
    gain = np.array([1, 1, W, H, W, H], np.float32)
    t = (targets * gain).astype(np.float32)
    anc = ANCHORS[lvl]
    with np.errstate(divide='ignore', invalid='ignore'):
        r = anc[:, None, :] / t[None, :, 4:6]
        bmask = np.max(np.maximum(r, 1.0 / r), axis=2) < ANCHOR_T   # [3, M]
    bmask = bmask & np.isfinite(t[:, 4:6]).all(1)[None, :]

    img = np.clip(targets[:, 0].astype(np.int32), 0, N_IMG - 1)
    cls_id = np.clip(targets[:, 1].astype(np.int32), 0, NCLS - 1)
    cx, cy = t[:, 2], t[:, 3]
    remx, remy = cx % 1.0, cy % 1.0
    gx0 = np.floor(cx).astype(np.int64)
    gy0 = np.floor(cy).astype(np.int64)

    sl_ok = np.stack([
        np.ones(M, bool),
        (remx < 0.5) & (cx > 1.0),
        (remy < 0.5) & (cy > 1.0),
        (remx > 0.5) & (cx < W - 1.0),
        (remy > 0.5) & (cy < H - 1.0),
    ])
    cellx = np.clip(gx0[None, :] + SLOT_D[:, 1][:, None], 0, W - 1)
    celly = np.clip(gy0[None, :] + SLOT_D[:, 0][:, None], 0, H - 1)
    offs = np.array([[0., 0.], [0.5, 0.], [0., 0.5], [-0.5, 0.], [0., -0.5]],
                    np.float32)
    offx = cx[None, :] - np.floor(cx[None, :] - offs[:, 0][:, None])
    offy = cy[None, :] - np.floor(cy[None, :] - offs[:, 1][:, None])
    return dict(H=H, W=W, bmask=bmask, img=img, cls_id=cls_id,
                tw=t[:, 4], th=t[:, 5], sl_ok=sl_ok, cellx=cellx,
                celly=celly, offx=offx, offy=offy, anc=anc)


class _Prep:
    """Builds the dense per-core device inputs + finalize metadata."""

    def __init__(self, targets, p_list):
        targets = np.asarray(targets, np.float32)
        cols = {k: [] for k in ('lvl', 'img', 'a', 'cy', 'cx', 'ox', 'oy',
                                'tw', 'th', 'cls')}
        rows_parts = []
        self.lv_sizes = []
        for lvl in range(3):
            L = _build_level(targets, lvl)
            aa, mm = np.nonzero(L['bmask'])
            n_lvl = 0
            e_img, e_a, e_cy, e_cx = [], [], [], []
            for s in range(5):
                sel = L['sl_ok'][s, mm]
                asel, msel = aa[sel], mm[sel]
                n = len(asel)
                n_lvl += n
                e_img.append(L['img'][msel])
                e_a.append(asel)
                e_cy.append(L['celly'][s, msel])
                e_cx.append(L['cellx'][s, msel])
                cols['ox'].append(L['offx'][s, msel])
                cols['oy'].append(L['offy'][s, msel])
                cols['tw'].append(L['tw'][msel])
                cols['th'].append(L['th'][msel])
                cols['cls'].append(L['cls_id'][msel])
                cols['lvl'].append(np.full(n, lvl, np.int64))
            e_img = np.concatenate(e_img)
            e_a = np.concatenate(e_a)
            e_cy = np.concatenate(e_cy)
            e_cx = np.concatenate(e_cx)
            cols['img'].append(e_img)
            cols['a'].append(e_a)
            cols['cy'].append(e_cy)
            cols['cx'].append(e_cx)
            self.lv_sizes.append(n_lvl)
            H, W = LEVEL_HW[lvl]
            pr = p_list[lvl].reshape(N_IMG, A, 5 + NCLS, H, W)
            rows_parts.append(pr[e_img, e_a, :, e_cy, e_cx])   # [n_lvl, 85]

        self.e = {k: np.concatenate(v) for k, v in cols.items()}
        rows = np.concatenate(rows_parts, axis=0)              # [ntot, 85]
        self.ntot = rows.shape[0]
        self.T = max(1, -(-self.ntot // (N_CORES * 128)))
        self.E = self.T * 128
        T = self.T

        e = self.e
        self.x_obj = rows[:, 4].astype(np.float64)
        self.x_tgt = rows[np.arange(self.ntot), 5 + e['cls']].astype(np.float64)
        anc2 = 2.0 * ANCHORS[e['lvl'], e['a']]                 # [ntot, 2]
        # +0.5 shift: device uses pxy = 2*sigma (not 2*sigma - 0.5); GIoU is
        # translation-invariant so the target corners absorb the shift.
        tc1 = np.stack([e['ox'] - e['tw'] * 0.5 + 0.5,
                        e['oy'] - e['th'] * 0.5 + 0.5], axis=1)
        tc2 = np.stack([e['ox'] + e['tw'] * 0.5 + 0.5,
                        e['oy'] + e['th'] * 0.5 + 0.5], axis=1)
        tarea = (e['tw'] * e['th'] + EPS)[:, None]

        self.negbox4 = self._pack(-rows[:, 0:4], 0.0).astype(BF16_NP)
        self.cls80 = self._pack(rows[:, 5:85], 0.0).astype(FP8_NP)
        rdp = [self._pack(tc1, 0.0), self._pack(tc2, 1.0),
               self._pack(anc2.astype(np.float32), 1.0),
               self._pack(tarea, 1.0)]
        self.rdp = np.concatenate(rdp, axis=2)                 # [8,128,7T]
        # out layout: [giou (T) | obj sums (3) | cls sums (T)]
        self.OUTW = 2 * T + 3

    def _pack(self, arr, pad_val):
        """[ntot, w] -> [8, 128, T*w]; entry j of core c at p=j%128,t=j//128."""
        w = arr.shape[1]
        full = np.full((N_CORES * self.E, w), pad_val, np.float32)
        full[:self.ntot] = arr
        return np.ascontiguousarray(
            full.reshape(N_CORES, self.T, 128, w).transpose(0, 2, 1, 3)
            .reshape(N_CORES, 128, self.T * w))

    def _unpack(self, dev_cols):
        """[8, 128, T] device outputs -> [ntot] in global entry order."""
        return (dev_cols.transpose(0, 2, 1).reshape(N_CORES * self.E)
                [:self.ntot].astype(np.float64))

    def build_objcls(self, p_list, c):
        """[obj plane | pad | cls] fp8 [128, OBJ_WP + 80T]."""
        objs = []
        for lvl in range(3):
            H, W = LEVEL_HW[lvl]
            p = p_list[lvl][c * IMG_PER_CORE:(c + 1) * IMG_PER_CORE]
            ob = np.ascontiguousarray(
                p.reshape(IMG_PER_CORE, A, 5 + NCLS, H, W)[:, :, 4]).reshape(-1)
            need = 128 * OBJ_COLS[lvl]
            if len(ob) < need:
                ob = np.concatenate(
                    [ob, np.full(need - len(ob), OBJ_PAD_VAL, np.float32)])
            objs.append(ob.reshape(128, OBJ_COLS[lvl]).astype(FP8_NP))
        objs.append(np.full((128, OBJ_WP - OBJ_W), OBJ_PAD_VAL,
                            np.float32).astype(FP8_NP))
        objs.append(self.cls80[c])
        return np.concatenate(objs, axis=1)

    def finalize(self, outs):
        T = self.T
        out3 = np.stack(outs)                                  # [8,128,2T+3]
        gp = self._unpack(out3[:, :, 0:T])                     # iou + un/ca
        cls_sum = self._unpack(out3[:, :, T + 3:2 * T + 3])
        e = self.e
        total = 0.0
        off = 0
        for lvl in range(3):
            n = self.lv_sizes[lvl]
            sl = slice(off, off + n)
            off += n
            H, W = LEVEL_HW[lvl]
            cnt = max(float(n), 1.0)
            lbox = np.sum(2.0 - gp[sl]) / cnt
            lcls = (np.sum(cls_sum[sl]) - np.sum(self.x_tgt[sl])) / (cnt * NCLS)
            s_obj = float(out3[:, :, T + lvl].sum(dtype=np.float64))
            # scatter-max dedup of clamped giou into objectness targets
            corr = 0.0
            if n:
                G = gp[sl] - 1.0
                fk = (((e['img'][sl] * A + e['a'][sl]) * H + e['cy'][sl]) * W
                      + e['cx'][sl])
                order = np.argsort(fk, kind='stable')
                fk_s = fk[order]
                vv = np.clip(G, 0.0, None)[order]
                xx = self.x_obj[sl][order]
                _, start = np.unique(fk_s, return_index=True)
                ymax = np.maximum.reduceat(vv, start)
                corr = np.sum(ymax * xx[start])
            count = N_IMG * A * H * W
            lobj = (s_obj - corr) / count
            total += (HYP_BOX * lbox + HYP_CLS * lcls
                      + HYP_OBJ * BALANCE[lvl] * lobj)
        return np.float32(total * N_IMG)


# --------------------------------------------------------------------------
# device kernel
# --------------------------------------------------------------------------

def _exp_ln_table_id(nc):
    tabs = get_activation_tables(nc.m.arch)
    act = mybir.ActivationFunctionType
    for i, funcs in enumerate(tabs.values()):
        if act.Exp in funcs and act.Ln in funcs:
            return i
    return None


def _build_bass(T):
    nc = bacc.Bacc('TRN2', debug=False, num_devices=N_CORES)
    BW = 4 * T + OBJ_W + 80 * T          # exp cols: negbox | obj | cls
    ob0 = 4 * T                          # obj slice start
    cb0 = ob0 + OBJ_W                    # cls slice start
    OCW = OBJ_WP + 80 * T
    nb_d = nc.dram_tensor('negbox', [128, 4 * T], BF16, kind='ExternalInput')
    oc_d = nc.dram_tensor('objcls', [128, OCW], FP8, kind='ExternalInput')
    rdp_d = nc.dram_tensor('rdp', [128, 7 * T], F32, kind='ExternalInput')
    out_d = nc.dram_tensor('out', [128, 2 * T + 3], F32, kind='ExternalOutput')

    with tile.TileContext(nc) as tc:
        with contextlib.ExitStack() as ctx:
            pool = ctx.enter_context(tc.tile_pool(name='sbuf', bufs=1))
            tt = mybir.AluOpType
            act = mybir.ActivationFunctionType

            nb_t = pool.tile([128, 4 * T], BF16)
            oc_t = pool.tile([128, OCW], FP8)
            # transfers serialize on the DMA engine in trigger order, so
            # issue box, then obj, then cls (ACT consumes in that order)
            nc.sync.dma_start(nb_t[:], nb_d.ap())
            nc.sync.dma_start(oc_t[:, 0:OBJ_WP], oc_d.ap()[:, 0:OBJ_WP])
            nc.sync.dma_start(oc_t[:, OBJ_WP:OCW], oc_d.ap()[:, OBJ_WP:OCW])
            rdp_t = pool.tile([128, 7 * T], F32)
            nc.gpsimd.dma_start(rdp_t[:], rdp_d.ap())
            out_t = pool.tile([128, 2 * T + 3], F32)

            tc1 = rdp_t[:, 0:2 * T]
            tc2 = rdp_t[:, 2 * T:4 * T]
            awh2 = rdp_t[:, 4 * T:6 * T]
            tarea = rdp_t[:, 6 * T:7 * T]

            # ---- scalar engine: preload the exp+ln table once, then
            # exp over the blob and ln(1+e) over the obj/cls slices.
            tab = _exp_ln_table_id(nc)
            if tab is not None:
                nc.scalar.add_instruction(mybir.InstLoadActFuncSet(
                    act_func_set_id=tab, name=nc.get_next_instruction_name(),
                    engine=mybir.EngineType.Activation, ins=[], outs=[]))
            pe = pool.tile([128, BW], BF16)
            nc.scalar.activation(pe[:, 0:ob0], nb_t[:], act.Exp)
            nc.scalar.activation(pe[:, ob0:cb0], oc_t[:, 0:OBJ_W], act.Exp)
            nc.scalar.activation(pe[:, cb0:BW], oc_t[:, OBJ_WP:OCW], act.Exp)
            lno = pool.tile([128, OBJ_W], BF16)
            nc.scalar.activation(lno[:], pe[:, ob0:cb0], act.Ln, bias=1.0)
            CH1 = 8 * 80                   # cls ln/reduce chunk split
            lnc = pool.tile([128, 80 * T], BF16)
            nc.scalar.activation(lnc[:, 0:CH1], pe[:, cb0:cb0 + CH1],
                                 act.Ln, bias=1.0)
            nc.scalar.activation(lnc[:, CH1:80 * T], pe[:, cb0 + CH1:BW],
                                 act.Ln, bias=1.0)

            # ---- vector engine: box sigmoid, GIoU chain, reductions
            def f32t(w, tag):
                return pool.tile([128, w], F32, name=tag, tag=tag)

            def xy(ap2):
                v = ap2.rearrange('p (c e) -> p c e', e=2)
                return v[:, :, 0], v[:, :, 1]

            sd = f32t(4 * T, 'sd')     # 1 + exp(-x)
            nc.vector.tensor_scalar_add(sd[:], pe[:, 0:ob0], 1.0)
            sig = f32t(4 * T, 'sig')
            nc.vector.reciprocal(sig[:], sd[:])
            sig4 = sig[:].rearrange('p (c e) -> p c e', e=4)
            sq = f32t(2 * T, 'sq')
            nc.vector.tensor_tensor(out=sq[:].rearrange('p (c e) -> p c e',
                                                        e=2),
                                    in0=sig4[:, :, 2:4], in1=sig4[:, :, 2:4],
                                    op=tt.mult)
            hwh = f32t(2 * T, 'hwh')   # pwh/2 = 2*anc*sig^2
            nc.vector.tensor_tensor(out=hwh[:], in0=sq[:], in1=awh2,
                                    op=tt.mult)
            # pxy = 2*sigma folded into both corner ops (host shifted tc +0.5)
            # bb = [b1|b2]; vs tc = [tc1|tc2]: max -> [i1|c2], min -> [c1|i2]
            sxy = sig4[:, :, 0:2]
            bb = f32t(4 * T, 'bb')
            nc.vector.scalar_tensor_tensor(out=bb[:, 0:2 * T].rearrange(
                                               'p (c e) -> p c e', e=2),
                                           in0=sxy, scalar=2.0,
                                           in1=hwh[:].rearrange(
                                               'p (c e) -> p c e', e=2),
                                           op0=tt.mult, op1=tt.subtract)
            nc.vector.scalar_tensor_tensor(out=bb[:, 2 * T:4 * T].rearrange(
                                               'p (c e) -> p c e', e=2),
                                           in0=sxy, scalar=2.0,
                                           in1=hwh[:].rearrange(
                                               'p (c e) -> p c e', e=2),
                                           op0=tt.mult, op1=tt.add)
            mx = f32t(4 * T, 'mx')     # [i1 | c2]
            nc.vector.tensor_tensor(out=mx[:], in0=bb[:], in1=rdp_t[:, 0:4 * T],
                                    op=tt.max)
            mn = f32t(4 * T, 'mn')     # [c1 | i2]
            nc.vector.tensor_tensor(out=mn[:], in0=bb[:], in1=rdp_t[:, 0:4 * T],
                                    op=tt.min)
            iw = f32t(2 * T, 'iw')
            nc.vector.tensor_tensor(out=iw[:], in0=mn[:, 2 * T:4 * T],
                                    in1=mx[:, 0:2 * T], op=tt.subtract)
            iwc = f32t(2 * T, 'iwc')
            nc.vector.tensor_scalar_max(iwc[:], iw[:], 0.0)
            iwx, iwy = xy(iwc[:])
            inter = f32t(T, 'inter')
            nc.vector.tensor_tensor(out=inter[:], in0=iwx, in1=iwy, op=tt.mult)
            hx, hy = xy(hwh[:])
            hp = f32t(T, 'hp')
            nc.vector.tensor_tensor(out=hp[:], in0=hx, in1=hy, op=tt.mult)
            u1 = f32t(T, 'u1')        # parea + tarea = 4*hp + tarea
            nc.vector.scalar_tensor_tensor(out=u1[:], in0=hp[:], scalar=4.0,
                                           in1=tarea, op0=tt.mult, op1=tt.add)
            un = f32t(T, 'un')
            nc.vector.tensor_tensor(out=un[:], in0=u1[:], in1=inter[:],
                                    op=tt.subtract)
            ru = f32t(T, 'ru')
            nc.vector.reciprocal(ru[:], un[:])
            iou = f32t(T, 'iou')
            nc.vector.tensor_tensor(out=iou[:], in0=inter[:], in1=ru[:],
                                    op=tt.mult)
            cwh = f32t(2 * T, 'cwh')
            nc.vector.tensor_tensor(out=cwh[:], in0=mx[:, 2 * T:4 * T],
                                    in1=mn[:, 0:2 * T], op=tt.subtract)
            cwx, cwy = xy(cwh[:])
            ca = f32t(T, 'ca')        # cw*ch (>0 strictly; eps dropped)
            nc.vector.tensor_tensor(out=ca[:], in0=cwx, in1=cwy, op=tt.mult)
            rc = f32t(T, 'rc')
            nc.vector.reciprocal(rc[:], ca[:])
            q = f32t(T, 'q')
            nc.vector.tensor_tensor(out=q[:], in0=un[:], in1=rc[:], op=tt.mult)
            # giou = iou - (ca-un)/ca = (iou + un/ca) - 1 ; host subtracts 1
            nc.vector.tensor_tensor(out=out_t[:, 0:T], in0=iou[:], in1=q[:],
                                    op=tt.add)

            # per-level objectness softplus sums -> out[:, T:T+3]
            o = 0
            for lvl in range(3):
                w = OBJ_COLS[lvl]
                nc.vector.reduce_sum(out_t[:, T + lvl:T + lvl + 1],
                                     lno[:, o:o + w], axis=mybir.AxisListType.X)
                o += w
            # first output: giou + obj sums, overlaps the cls tail
            nc.sync.dma_start(out_d.ap()[:, 0:T + 3], out_t[:, 0:T + 3])

            # per-entry cls softplus sums -> out[:, T+3:2T+3]
            nc.vector.reduce_sum(
                out_t[:, T + 3:T + 3 + CH1 // 80],
                lnc[:, 0:CH1].rearrange('p (b e) -> p b e', e=80),
                axis=mybir.AxisListType.X)
            nc.vector.reduce_sum(
                out_t[:, T + 3 + CH1 // 80:2 * T + 3],
                lnc[:, CH1:80 * T].rearrange('p (b e) -> p b e', e=80),
                axis=mybir.AxisListType.X)
            nc.sync.dma_start(out_d.ap()[:, T + 3:2 * T + 3],
                              out_t[:, T + 3:2 * T + 3])
    nc.compile()
    return nc


# --------------------------------------------------------------------------
# entry point
# --------------------------------------------------------------------------

def kernel(p0, p1, p2, targets):
    p0 = np.asarray(p0, np.float32)
    p1 = np.asarray(p1, np.float32)
    p2 = np.asarray(p2, np.float32)
    targets = np.asarray(targets, np.float32)
    p_list = [p0, p1, p2]
    prep = _Prep(targets, p_list)
    nc = _build_bass(prep.T)

    in_maps = []
    for c in range(N_CORES):
        in_maps.append({
            'negbox': prep.negbox4[c],
            'objcls': prep.build_objcls(p_list, c),
            'rdp': prep.rdp[c],
        })
    res = bass_utils.run_bass_kernel_spmd(nc, in_maps,
                                          core_ids=list(range(N_CORES)))
    global LAST_EXEC_NS, LAST_RESULT
    LAST_EXEC_NS = res.exec_time_ns
    LAST_RESULT = res
    outs = [res.results[c]['out'] for c in range(N_CORES)]
    return np.asarray(prep.finalize(outs), np.float32)


LAST_EXEC_NS = None
LAST_RESULT = None
